# revision 28
# baseline (speedup 1.0000x reference)
"""Trainium2 Bass kernel for nn_MultiHeadAttentionLayer (GNN message passing).

Contract: kernel(**inputs) takes the FULL unsharded inputs (as produced by
setup_inputs()) and returns the FULL output [N, H, D] float32.

Strategy (8 NeuronCores, SPMD, no collectives):
  - dst == arange(E) % N, so node n receives exactly 8 edges: e = k*N + n.
    Shard destination nodes across cores; each core owns N/8 nodes and all 8
    incoming edges per node.  The segment_sum becomes a dense 8-step
    accumulation into PSUM - no scatter.
  - The host pre-gathers each (node-block, edge-slice)'s src h rows into a
    dim-major DRAM chunk ([128 dims, 2 chunks, 640 edges]) - exactly the
    lhsT layout PE wants.  The device just STREAMS these contiguous chunks
    with plain HWDGE dma_start: no dma_gather, no GPSIMD descriptor
    generation (the previous version's 80 dma_gathers serialized on the
    Pool engine at ~5.8us each = ~465us of the 518us exec time).  K|V are
    projected on the fly (2 matmuls per m-tile) straight into PSUM.
  - ACT drains K|V PSUM to bf16 SBUF; DVE computes the dot-product score
    (2x-mode bf16 multiply + reduce), clip, and the score*V scaling.  The V
    columns are host-permuted to d*8+h (head-minor) so score+jaccard scaling
    is a single packed 2x-eligible multiply against an [se|jac] vector.
  - Identity-matmuls accumulate the 8 slices into PSUM; final divide by z.

Host-side work is limited to layout (transpose/pad/unique/renumber/permute)
and the final concatenation; all FLOPs happen on device.
"""

import sys

import numpy as np

for _p in ("/opt/trn_rl_repo",):
    if _p not in sys.path:
        sys.path.insert(0, _p)

# --- problem constants (hardcoded per spec; kernel.py must be self-contained)
N_NODES = 50000
N_EDGES = 400000
IN_DIM = 256
OUT_DIM = 32
NUM_HEADS = 8
H2 = NUM_HEADS // 2
N_CORES = 8
P = 128

CLIP = 5.0 * np.sqrt(np.float32(32.0))  # clip on the raw dot product
SCALE = float(1.0 / np.sqrt(np.float32(32.0)))


class Cfg:
    def __init__(self, n_nodes=N_NODES, n_edges=N_EDGES, n_cores=N_CORES):
        assert n_edges == 8 * n_nodes
        self.N = n_nodes
        self.E = n_edges
        self.NC = n_cores
        assert n_nodes % n_cores == 0
        self.NPC = n_nodes // n_cores          # nodes per core
        self.BM = 5                            # m-tiles per block
        m = -(-self.NPC // P)                  # minimal 128-node tiles
        self.M = -(-m // self.BM) * self.BM    # padded to block multiple
        self.NPAD = self.M * P                 # padded nodes per core
        self.NB = self.M // self.BM            # blocks
        self.EB = self.BM * P                  # edges per (block, slice)


FULL_CFG = Cfg()


# --------------------------------------------------------------------------
# device program
# --------------------------------------------------------------------------

def build_program(cfg: Cfg, repeat: int = 1, ablate: str = ""):
    import concourse.bacc as bacc
    import concourse.mybir as mybir
    import concourse.tile as tile
    from concourse.masks import make_identity

    f32 = mybir.dt.float32
    bf16 = mybir.dt.bfloat16
    M, BM, NB, EB = cfg.M, cfg.BM, cfg.NB, cfg.EB

    nc = bacc.Bacc(
        "TRN2",
        target_bir_lowering=False,
        debug=False,
        enable_asserts=False,
        num_devices=cfg.NC,
    )

    h_edges = nc.dram_tensor(
        "h_edges", [NB * 9, P, 2 * EB], bf16, kind="ExternalInput")
    w_kv = nc.dram_tensor("w_kv", [P, 2, 384], bf16, kind="ExternalInput")
    w_q = nc.dram_tensor("w_q", [P, 2, P], bf16, kind="ExternalInput")
    jac_t = nc.dram_tensor("jac", [P, 8 * M], bf16, kind="ExternalInput")
    out_t = nc.dram_tensor("out", [P, M * 256], f32, kind="ExternalOutput")

    mult = mybir.AluOpType.mult
    add = mybir.AluOpType.add

    with tile.TileContext(nc) as tc:
        with (
            tc.tile_pool(name="const", bufs=1) as const,
        ):
          for _rep in range(repeat):
            wkv_sb = const.tile([P, 2, 384], bf16)
            wq_sb = const.tile([P, 2, P], bf16)
            jac_sb = const.tile([P, 8 * M], bf16)
            qloc = const.tile([P, M, P], bf16)
            ident = const.tile([P, P], bf16)

            nc.sync.dma_start(out=wkv_sb[:], in_=w_kv[:])
            nc.sync.dma_start(out=wq_sb[:], in_=w_q[:])
            nc.sync.dma_start(out=jac_sb[:], in_=jac_t[:])
            make_identity(nc, ident[:])

            # ---- streamed blocks: round 0 projects Q of the block's own
            # dst nodes (their h rows are chunk b*9+0 of h_edges), rounds
            # 1..8 are the 8 edge slices.  Everything flows through the same
            # kv2/kv3 PSUM ring - no serial Q prologue.
            with (
                tc.tile_pool(name="pg", bufs=3) as pg,
                tc.tile_pool(name="kvp", bufs=5, space="PSUM") as kvp,
                tc.tile_pool(name="accp", bufs=1, space="PSUM") as accp,
                tc.tile_pool(name="sp", bufs=2) as sp,
                tc.tile_pool(name="po", bufs=2) as po,
            ):
                for b in range(NB):
                    acc = accp.tile([P, BM, 256], f32, tag="acc")
                    # per-block score|jaccard table; jaccard halves prefilled
                    # for all 8 slices in one strided copy
                    sej = sp.tile([P, 8, BM, 8], bf16, tag="sej", bufs=2)
                    jb = jac_sb[:].rearrange(
                        "p (k m) -> p k m", k=8)[:, :, b * BM:(b + 1) * BM]
                    ("nov" in ablate) or nc.gpsimd.tensor_copy(
                        out=sej[:, :, :, 4:8],
                        in_=jb.unsqueeze(3).to_broadcast([P, 8, BM, 4]))
                    for r in range(9):
                        k = r - 1
                        gathT = pg.tile([P, 2, EB], bf16, tag="g", bufs=6)
                        ("nog" in ablate) or nc.sync.dma_start(
                            out=gathT[:].rearrange("p c e -> p (c e)"),
                            in_=h_edges[b * 9 + r, :, :],
                        )
                        # on-the-fly projection; two PSUM tiles (2+3 m-tiles)
                        # so the ACT drains are 2 big copies, not 5.  512-f32
                        # stride per m keeps each matmul bank-aligned.
                        kv2 = kvp.tile([P, 2, 512], f32, tag="kv2", bufs=1)
                        kv3 = kvp.tile([P, 3, 512], f32, tag="kv3", bufs=1)
                        w_sb, wid = (wq_sb, P) if r == 0 else (wkv_sb, 384)
                        for m in range(BM):
                            kv = kv2[:, m, 0:wid] if m < 2 else kv3[:, m - 2, 0:wid]
                            ("nom" in ablate) or nc.tensor.matmul(
                                out=kv,
                                lhsT=gathT[:, 0, m * P:(m + 1) * P],
                                rhs=w_sb[:, 0, :], start=True, stop=False,
                            )
                            ("nom" in ablate) or nc.tensor.matmul(
                                out=kv,
                                lhsT=gathT[:, 1, m * P:(m + 1) * P],
                                rhs=w_sb[:, 1, :], start=False, stop=True,
                            )
                        if r == 0:
                            # drain Q of this block's dst nodes
                            nc.scalar.copy(
                                out=qloc[:, b * BM:b * BM + 2, :],
                                in_=kv2[:, :, 0:P])
                            nc.scalar.copy(
                                out=qloc[:, b * BM + 2:(b + 1) * BM, :],
                                in_=kv3[:, :, 0:P])
                            continue
                        j = k % 2
                        if j == 0:
                            kvb = sp.tile([P, 2, BM, 384], bf16, tag="kvb",
                                          bufs=3)
                        nc.scalar.copy(out=kvb[:, j, 0:2, :],
                                       in_=kv2[:, :, 0:384])
                        nc.scalar.copy(out=kvb[:, j, 2:BM, :],
                                       in_=kv3[:, :, 0:384])
                        if j == 0:
                            continue
                        # ---- score both slices of the pair in one batch ----
                        k0 = k - 1
                        prod = sp.tile([P, 2, BM, P], bf16, tag="prod", bufs=2)
                        ("nov" in ablate) or nc.vector.tensor_tensor(
                            out=prod[:], in0=kvb[:, :, :, 0:P],
                            in1=qloc[:, b * BM:(b + 1) * BM, :].unsqueeze(
                                1).to_broadcast([P, 2, BM, P]), op=mult)
                        dot = sp.tile([P, 2, BM, 4], f32, tag="dot", bufs=2)
                        ("nov" in ablate) or nc.vector.tensor_reduce(
                            out=dot[:],
                            in_=prod[:].rearrange(
                                "p s m (h f) -> p (s m) h f", f=32),
                            axis=mybir.AxisListType.X, op=add,
                        )
                        ("nov" in ablate) or nc.vector.tensor_scalar(
                            out=dot[:], in0=dot[:], scalar1=float(CLIP),
                            scalar2=float(-CLIP),
                            op0=mybir.AluOpType.min, op1=mybir.AluOpType.max,
                        )
                        ("nov" in ablate) or nc.scalar.activation(
                            out=sej[:, k0:k + 1, :, 0:4], in_=dot[:],
                            func=mybir.ActivationFunctionType.Exp, scale=SCALE,
                        )
                        # scale V rows: V is d*8+h interleaved, so one packed
                        # multiply against the 8-wide [se|jac] vector
                        scl = sp.tile([P, 2, BM, 256], bf16, tag="scl", bufs=3)
                        ("nov" in ablate) or nc.vector.tensor_tensor(
                            out=scl[:].rearrange("p s m (f c) -> p (s m) f c",
                                                 c=8),
                            in0=kvb[:, :, :, P:384].rearrange(
                                "p s m (f c) -> p (s m) f c", c=8),
                            in1=sej[:, k0:k + 1].rearrange(
                                "p s m h -> p (s m) h").unsqueeze(
                                2).to_broadcast([P, 2 * BM, 32, 8]),
                            op=mult)
                        # accumulate the pair into PSUM (512-col chunks)
                        for jj in range(2):
                            sclf = scl[:, jj].rearrange("p m f -> p (m f)")
                            for c0, c1 in ((0, 512), (512, 1024),
                                           (1024, 1280)):
                                ("nom" in ablate) or nc.tensor.matmul(
                                    out=acc[:].rearrange(
                                        "p m f -> p (m f)")[:, c0:c1],
                                    lhsT=ident[:], rhs=sclf[:, c0:c1],
                                    start=(k0 + jj == 0), stop=(k0 + jj == 7),
                                )
    # finalize block b: z = sum_k(sej) in one strided reduce,
                    # then out = acc * (1/z) on DVE
                    zm = sp.tile([P, BM, 8], f32, tag="zm", bufs=2)
                    ("nov" in ablate) or nc.vector.tensor_reduce(
                        out=zm[:], in_=sej[:].rearrange("p k m h -> p m h k"),
                        axis=mybir.AxisListType.X, op=add)
                    zrf = sp.tile([P, BM, 8], f32, tag="zrf", bufs=2)
                    nc.vector.reciprocal(zrf[:], zm[:])
                    ost = po.tile([P, BM, 256], f32, tag="ost", bufs=3)
                    nc.vector.tensor_tensor(
                        out=ost[:].rearrange("p m (f c) -> p m f c", c=8),
                        in0=acc[:].rearrange("p m (f c) -> p m f c", c=8),
                        in1=zrf[:].unsqueeze(2).to_broadcast([P, BM, 32, 8]),
                        op=mult)
                    nc.sync.dma_start(
                        out=out_t[:, b * BM * 256:(b + 1) * BM * 256],
                        in_=ost[:].rearrange("p m f -> p (m f)"))

    nc.compile()
    return nc


# --------------------------------------------------------------------------
# host-side sharding / assembly
# --------------------------------------------------------------------------

def host_prepare(cfg: Cfg, h, src, jaccard, Wq, Wk, Wv):
    """Build the per-core input maps (layout only, no FLOPs)."""
    import ml_dtypes

    f32 = np.float32
    bf16 = ml_dtypes.bfloat16
    hT = np.ascontiguousarray(h.T.astype(f32, copy=False)).astype(bf16)

    def chunk2(w_t, width):  # [256, width] -> [128, 2, width]
        return np.ascontiguousarray(
            w_t.reshape(2, P, width).transpose(1, 0, 2)).astype(bf16)

    # V columns permuted to d*8+h (head-minor) for the packed scale multiply
    perm_v = np.arange(256).reshape(NUM_HEADS, OUT_DIM).T.reshape(-1)
    wv_t = np.ascontiguousarray(Wv.T.astype(f32))[:, perm_v]
    w_kv = chunk2(np.concatenate([Wk.T.astype(f32), wv_t], axis=1), 384)
    w_q = chunk2(np.ascontiguousarray(Wq.T.astype(f32)), P)

    # slot s = m*128 + p  <->  node n = c*NPC + s  (s < NPC valid)
    s_grid = np.arange(cfg.M)[None, :] * P + np.arange(P)[:, None]  # [128, M]
    valid = s_grid < cfg.NPC

    in_maps = []
    for c in range(cfg.NC):
        base = c * cfg.NPC
        src_mat = np.zeros((8, cfg.NPAD), dtype=np.int64)
        jac = np.ones((P, 8 * cfg.M), dtype=f32)
        for k in range(8):
            e = k * cfg.N + base + np.clip(s_grid, 0, cfg.NPC - 1)
            sv = np.where(valid, src[e], 0)                # [128, M]; 0 pad
            src_mat[k] = sv.T.reshape(-1)                  # slot order m*128+p
            jac[:, k * cfg.M:(k + 1) * cfg.M] = np.where(valid, jaccard[e], 1.0)
        # dim-major pre-gathered h rows.  Round r=0 of block b carries the
        # block's OWN dst-node rows (for the Q projection); rounds 1..8 are
        # the 8 edge slices: h_edges[b*9+r, p, ch*EB+sl] =
        #   hT[ch*128+p, (own|src_mat[r-1])[b*EB+sl]]
        own = np.clip(base + np.arange(cfg.NPAD), 0, cfg.N - 1)  # [NPAD]
        rows = np.concatenate([own[None, :], src_mat], axis=0)   # [9, NPAD]
        G = hT[:, rows]                                    # [256, 9, NPAD]
        G = G.reshape(2, P, 9, cfg.NB, cfg.EB)             # [ch, p, r, b, sl]
        h_edges = np.ascontiguousarray(
            G.transpose(3, 2, 1, 0, 4)).reshape(cfg.NB * 9, P, 2 * cfg.EB)

        in_maps.append({
            "h_edges": h_edges,
            "w_kv": w_kv,
            "w_q": w_q,
            "jac": jac.astype(bf16),
        })
    return in_maps


def assemble_output(cfg: Cfg, results):
    out = np.empty((cfg.N, NUM_HEADS, OUT_DIM), dtype=np.float32)
    for c, r in enumerate(results):
        shard = r["out"].reshape(P, cfg.M, 256).transpose(1, 0, 2).reshape(
            cfg.NPAD, 256)[:cfg.NPC]
        # un-permute: column d*8+h holds (head h, dim d)
        out[c * cfg.NPC:(c + 1) * cfg.NPC] = shard.reshape(
            cfg.NPC, OUT_DIM, NUM_HEADS).transpose(0, 2, 1)
    return out


# --------------------------------------------------------------------------
# numpy fallback (used only if inputs don't match the spec'd structure)
# --------------------------------------------------------------------------

def _numpy_reference(h, src, dst, jaccard, Wq, bq, Wk, bk, Wv, bv):
    N = h.shape[0]
    E = src.shape[0]
    h = h.astype(np.float32)
    Qh = (h @ Wq.T + bq).reshape(N, H2, OUT_DIM)
    Kh = (h @ Wk.T + bk).reshape(N, H2, OUT_DIM)
    Vh = (h @ Wv.T + bv).reshape(N, NUM_HEADS, OUT_DIM)
    score = np.sum(Kh[src] * Qh[dst], axis=-1, keepdims=True)
    score = np.exp(np.clip(score / np.sqrt(np.float32(OUT_DIM)), -5.0, 5.0))
    jac = np.broadcast_to(jaccard[:, None, None], (E, H2, 1))
    score_new = np.concatenate([score, jac], axis=1).astype(np.float32)
    contrib = (Vh[src] * score_new).astype(np.float32)
    wV = np.zeros((N, NUM_HEADS, OUT_DIM), dtype=np.float32)
    z = np.zeros((N, NUM_HEADS, 1), dtype=np.float32)
    np.add.at(wV, dst, contrib)
    np.add.at(z, dst, score_new)
    return wV / z


# --------------------------------------------------------------------------
# entry point
# --------------------------------------------------------------------------

_PROGRAM_CACHE = {}


def _get_program(cfg: Cfg):
    key = (cfg.N, cfg.E, cfg.NC)
    if key not in _PROGRAM_CACHE:
        _PROGRAM_CACHE[key] = build_program(cfg)
    return _PROGRAM_CACHE[key]


def _structure_ok(h, src, dst, jaccard, Wq, bq, Wk, bk, Wv, bv):
    if h.shape != (N_NODES, IN_DIM) or src.shape != (N_EDGES,):
        return False
    if Wq.shape != (H2 * OUT_DIM, IN_DIM) or Wv.shape != (NUM_HEADS * OUT_DIM, IN_DIM):
        return False
    if np.any(bq) or np.any(bk) or np.any(bv):
        return False
    if not np.array_equal(
            np.asarray(dst, dtype=np.int64),
            np.arange(N_EDGES, dtype=np.int64) % N_NODES):
        return False
    if src.min() < 0 or src.max() >= N_NODES:
        return False
    return True


def run_on_hw(inputs):
    from concourse.bass2jax import run_bass_via_pjrt

    cfg = FULL_CFG
    nc = _get_program(cfg)
    in_maps = host_prepare(
        cfg, inputs["h"], inputs["src"], inputs["jaccard"],
        inputs["Wq"], inputs["Wk"], inputs["Wv"])
    results = run_bass_via_pjrt(nc, in_maps, n_cores=cfg.NC)
    return assemble_output(cfg, results), results


def kernel(**inputs) -> np.ndarray:
    args = {k: np.asarray(v) for k, v in inputs.items()}
    if not _structure_ok(**args):
        return _numpy_reference(**args)
    out, _ = run_on_hw(args)
    return out


if __name__ == "__main__":
    print("building full program...")
    nc = _get_program(FULL_CFG)
    print("ok")



# revision 29
# speedup vs baseline: 1.0722x; 1.0722x over previous
"""Trainium2 Bass kernel for nn_MultiHeadAttentionLayer (GNN message passing).

Contract: kernel(**inputs) takes the FULL unsharded inputs (as produced by
setup_inputs()) and returns the FULL output [N, H, D] float32.

Strategy (8 NeuronCores, SPMD, no collectives):
  - dst == arange(E) % N, so node n receives exactly 8 edges: e = k*N + n.
    Shard destination nodes across cores; each core owns N/8 nodes and all 8
    incoming edges per node.  The segment_sum becomes a dense 8-step
    accumulation into PSUM - no scatter.
  - The host pre-gathers each (node-block, edge-slice)'s src h rows into a
    dim-major DRAM chunk ([128 dims, 2 chunks, 640 edges]) - exactly the
    lhsT layout PE wants.  The device just STREAMS these contiguous chunks
    with plain HWDGE dma_start: no dma_gather, no GPSIMD descriptor
    generation (the previous version's 80 dma_gathers serialized on the
    Pool engine at ~5.8us each = ~465us of the 518us exec time).  K|V are
    projected on the fly (2 matmuls per m-tile) straight into PSUM.
  - ACT drains K|V PSUM to bf16 SBUF; DVE computes the dot-product score
    (2x-mode bf16 multiply + reduce), clip, and the score*V scaling.  The V
    columns are host-permuted to d*8+h (head-minor) so score+jaccard scaling
    is a single packed 2x-eligible multiply against an [se|jac] vector.
  - Identity-matmuls accumulate the 8 slices into PSUM; final divide by z.

Host-side work is limited to layout (transpose/pad/unique/renumber/permute)
and the final concatenation; all FLOPs happen on device.
"""

import sys

import numpy as np

for _p in ("/opt/trn_rl_repo",):
    if _p not in sys.path:
        sys.path.insert(0, _p)

# --- problem constants (hardcoded per spec; kernel.py must be self-contained)
N_NODES = 50000
N_EDGES = 400000
IN_DIM = 256
OUT_DIM = 32
NUM_HEADS = 8
H2 = NUM_HEADS // 2
N_CORES = 8
P = 128

CLIP = 5.0 * np.sqrt(np.float32(32.0))  # clip on the raw dot product
SCALE = float(1.0 / np.sqrt(np.float32(32.0)))


class Cfg:
    def __init__(self, n_nodes=N_NODES, n_edges=N_EDGES, n_cores=N_CORES):
        assert n_edges == 8 * n_nodes
        self.N = n_nodes
        self.E = n_edges
        self.NC = n_cores
        assert n_nodes % n_cores == 0
        self.NPC = n_nodes // n_cores          # nodes per core
        self.BM = 5                            # m-tiles per block
        m = -(-self.NPC // P)                  # minimal 128-node tiles
        self.M = -(-m // self.BM) * self.BM    # padded to block multiple
        self.NPAD = self.M * P                 # padded nodes per core
        self.NB = self.M // self.BM            # blocks
        self.EB = self.BM * P                  # edges per (block, slice)


FULL_CFG = Cfg()


# --------------------------------------------------------------------------
# device program
# --------------------------------------------------------------------------

def build_program(cfg: Cfg, repeat: int = 1, ablate: str = ""):
    import concourse.bacc as bacc
    import concourse.mybir as mybir
    import concourse.tile as tile
    from concourse.masks import make_identity

    f32 = mybir.dt.float32
    bf16 = mybir.dt.bfloat16
    M, BM, NB, EB = cfg.M, cfg.BM, cfg.NB, cfg.EB

    nc = bacc.Bacc(
        "TRN2",
        target_bir_lowering=False,
        debug=False,
        enable_asserts=False,
        num_devices=cfg.NC,
    )

    h_edges = nc.dram_tensor(
        "h_edges", [NB * 9, P, 2 * EB], bf16, kind="ExternalInput")
    w_kv = nc.dram_tensor("w_kv", [P, 2, 384], bf16, kind="ExternalInput")
    w_q = nc.dram_tensor("w_q", [P, 2, P], bf16, kind="ExternalInput")
    jac_t = nc.dram_tensor("jac", [P, 8 * M], bf16, kind="ExternalInput")
    out_t = nc.dram_tensor("out", [P, M * 256], f32, kind="ExternalOutput")

    mult = mybir.AluOpType.mult
    add = mybir.AluOpType.add

    with tile.TileContext(nc) as tc:
        with (
            tc.tile_pool(name="const", bufs=1) as const,
        ):
          for _rep in range(repeat):
            wkv_sb = const.tile([P, 2, 384], bf16)
            wq_sb = const.tile([P, 2, P], bf16)
            jac_sb = const.tile([P, 8 * M], bf16)
            qloc = const.tile([P, M, P], bf16)
            ident = const.tile([P, P], bf16)

            nc.sync.dma_start(out=wkv_sb[:], in_=w_kv[:])
            nc.sync.dma_start(out=wq_sb[:], in_=w_q[:])
            nc.sync.dma_start(out=jac_sb[:], in_=jac_t[:])
            make_identity(nc, ident[:])

            # ---- streamed blocks: round 0 projects Q of the block's own
            # dst nodes (their h rows are chunk b*9+0 of h_edges), rounds
            # 1..8 are the 8 edge slices.  Everything flows through the same
            # kv2/kv3 PSUM ring - no serial Q prologue.
            with (
                tc.tile_pool(name="pg", bufs=3) as pg,
                tc.tile_pool(name="kvp", bufs=5, space="PSUM") as kvp,
                tc.tile_pool(name="accp", bufs=1, space="PSUM") as accp,
                tc.tile_pool(name="sp", bufs=2) as sp,
                tc.tile_pool(name="po", bufs=2) as po,
            ):
                for b in range(NB):
                    acc = accp.tile([P, BM, 256], f32, tag="acc")
                    # per-block score|jaccard table; jaccard halves prefilled
                    # for all 8 slices in one strided copy
                    sej = sp.tile([P, 8, BM, 8], bf16, tag="sej", bufs=2)
                    jb = jac_sb[:].rearrange(
                        "p (k m) -> p k m", k=8)[:, :, b * BM:(b + 1) * BM]
                    ("nov" in ablate) or nc.gpsimd.tensor_copy(
                        out=sej[:, :, :, 4:8],
                        in_=jb.unsqueeze(3).to_broadcast([P, 8, BM, 4]))
                    for r in range(9):
                        k = r - 1
                        gathT = pg.tile([P, 2, EB], bf16, tag="g", bufs=6)
                        ("nog" in ablate) or nc.sync.dma_start(
                            out=gathT[:].rearrange("p c e -> p (c e)"),
                            in_=h_edges[b * 9 + r, :, :],
                        )
                        # on-the-fly projection; two PSUM tiles (2+3 m-tiles)
                        # so the ACT drains are 2 big copies, not 5.  512-f32
                        # stride per m keeps each matmul bank-aligned.
                        kv2 = kvp.tile([P, 2, 512], f32, tag="kv2", bufs=1)
                        kv3 = kvp.tile([P, 3, 512], f32, tag="kv3", bufs=1)
                        w_sb, wid = (wq_sb, P) if r == 0 else (wkv_sb, 384)
                        for m in range(BM):
                            kv = kv2[:, m, 0:wid] if m < 2 else kv3[:, m - 2, 0:wid]
                            ("nom" in ablate) or nc.tensor.matmul(
                                out=kv,
                                lhsT=gathT[:, 0, m * P:(m + 1) * P],
                                rhs=w_sb[:, 0, :], start=True, stop=False,
                            )
                            ("nom" in ablate) or nc.tensor.matmul(
                                out=kv,
                                lhsT=gathT[:, 1, m * P:(m + 1) * P],
                                rhs=w_sb[:, 1, :], start=False, stop=True,
                            )
                        if r == 0:
                            # drain Q of this block's dst nodes
                            nc.scalar.copy(
                                out=qloc[:, b * BM:b * BM + 2, :],
                                in_=kv2[:, :, 0:P])
                            nc.scalar.copy(
                                out=qloc[:, b * BM + 2:(b + 1) * BM, :],
                                in_=kv3[:, :, 0:P])
                            continue
                        kvb = sp.tile([P, BM, 384], bf16, tag="kvb", bufs=4)
                        nc.scalar.copy(out=kvb[:, 0:2, :], in_=kv2[:, :, 0:384])
                        nc.scalar.copy(out=kvb[:, 2:BM, :], in_=kv3[:, :, 0:384])
                        # score: dot(K, Q) over each head's 32 dims
                        prod = sp.tile([P, BM, P], bf16, tag="prod", bufs=4)
                        ("nov" in ablate) or nc.vector.tensor_tensor(
                            out=prod[:], in0=kvb[:, :, 0:P],
                            in1=qloc[:, b * BM:(b + 1) * BM, :], op=mult)
                        dot = sp.tile([P, BM, 4], f32, tag="dot", bufs=4)
                        ("nov" in ablate) or nc.vector.tensor_reduce(
                            out=dot[:],
                            in_=prod[:].rearrange("p m (h f) -> p m h f", f=32),
                            axis=mybir.AxisListType.X, op=add,
                        )
                        ("nov" in ablate) or nc.vector.tensor_scalar(
                            out=dot[:], in0=dot[:], scalar1=float(CLIP),
                            scalar2=float(-CLIP),
                            op0=mybir.AluOpType.min, op1=mybir.AluOpType.max,
                        )
                        ("nov" in ablate) or nc.scalar.activation(
                            out=sej[:, k, :, 0:4], in_=dot[:],
                            func=mybir.ActivationFunctionType.Exp, scale=SCALE,
                        )
                        # scale V rows: V is d*8+h interleaved, so one packed
                        # multiply against the 8-wide [se|jac] vector
                        scl = sp.tile([P, BM, 256], bf16, tag="scl", bufs=6)
                        ("nov" in ablate) or nc.vector.tensor_tensor(
                            out=scl[:].rearrange("p m (f c) -> p m f c", c=8),
                            in0=kvb[:, :, P:384].rearrange(
                                "p m (f c) -> p m f c", c=8),
                            in1=sej[:, k].unsqueeze(2).to_broadcast(
                                [P, BM, 32, 8]),
                            op=mult)
                        # accumulate the 8 slices into PSUM (512-col chunks)
                        sclf = scl[:].rearrange("p m f -> p (m f)")
                        for c0, c1 in ((0, 512), (512, 1024), (1024, 1280)):
                            ("nom" in ablate) or nc.tensor.matmul(
                                out=acc[:].rearrange(
                                    "p m f -> p (m f)")[:, c0:c1],
                                lhsT=ident[:], rhs=sclf[:, c0:c1],
                                start=(k == 0), stop=(k == 7),
                            )
    # finalize block b: z = sum_k(sej) in one strided reduce,
                    # then out = acc * (1/z) on DVE
                    zm = sp.tile([P, BM, 8], f32, tag="zm", bufs=2)
                    ("nov" in ablate) or nc.vector.tensor_reduce(
                        out=zm[:], in_=sej[:].rearrange("p k m h -> p m h k"),
                        axis=mybir.AxisListType.X, op=add)
                    zrf = sp.tile([P, BM, 8], f32, tag="zrf", bufs=2)
                    nc.vector.reciprocal(zrf[:], zm[:])
                    ost = po.tile([P, BM, 256], f32, tag="ost", bufs=3)
                    nc.vector.tensor_tensor(
                        out=ost[:].rearrange("p m (f c) -> p m f c", c=8),
                        in0=acc[:].rearrange("p m (f c) -> p m f c", c=8),
                        in1=zrf[:].unsqueeze(2).to_broadcast([P, BM, 32, 8]),
                        op=mult)
                    nc.sync.dma_start(
                        out=out_t[:, b * BM * 256:(b + 1) * BM * 256],
                        in_=ost[:].rearrange("p m f -> p (m f)"))

    nc.compile()
    return nc


# --------------------------------------------------------------------------
# host-side sharding / assembly
# --------------------------------------------------------------------------

def host_prepare(cfg: Cfg, h, src, jaccard, Wq, Wk, Wv):
    """Build the per-core input maps (layout only, no FLOPs)."""
    import ml_dtypes

    f32 = np.float32
    bf16 = ml_dtypes.bfloat16
    hT = np.ascontiguousarray(h.T.astype(f32, copy=False)).astype(bf16)

    def chunk2(w_t, width):  # [256, width] -> [128, 2, width]
        return np.ascontiguousarray(
            w_t.reshape(2, P, width).transpose(1, 0, 2)).astype(bf16)

    # V columns permuted to d*8+h (head-minor) for the packed scale multiply
    perm_v = np.arange(256).reshape(NUM_HEADS, OUT_DIM).T.reshape(-1)
    wv_t = np.ascontiguousarray(Wv.T.astype(f32))[:, perm_v]
    w_kv = chunk2(np.concatenate([Wk.T.astype(f32), wv_t], axis=1), 384)
    w_q = chunk2(np.ascontiguousarray(Wq.T.astype(f32)), P)

    # slot s = m*128 + p  <->  node n = c*NPC + s  (s < NPC valid)
    s_grid = np.arange(cfg.M)[None, :] * P + np.arange(P)[:, None]  # [128, M]
    valid = s_grid < cfg.NPC

    in_maps = []
    for c in range(cfg.NC):
        base = c * cfg.NPC
        src_mat = np.zeros((8, cfg.NPAD), dtype=np.int64)
        jac = np.ones((P, 8 * cfg.M), dtype=f32)
        for k in range(8):
            e = k * cfg.N + base + np.clip(s_grid, 0, cfg.NPC - 1)
            sv = np.where(valid, src[e], 0)                # [128, M]; 0 pad
            src_mat[k] = sv.T.reshape(-1)                  # slot order m*128+p
            jac[:, k * cfg.M:(k + 1) * cfg.M] = np.where(valid, jaccard[e], 1.0)
        # dim-major pre-gathered h rows.  Round r=0 of block b carries the
        # block's OWN dst-node rows (for the Q projection); rounds 1..8 are
        # the 8 edge slices: h_edges[b*9+r, p, ch*EB+sl] =
        #   hT[ch*128+p, (own|src_mat[r-1])[b*EB+sl]]
        own = np.clip(base + np.arange(cfg.NPAD), 0, cfg.N - 1)  # [NPAD]
        rows = np.concatenate([own[None, :], src_mat], axis=0)   # [9, NPAD]
        G = hT[:, rows]                                    # [256, 9, NPAD]
        G = G.reshape(2, P, 9, cfg.NB, cfg.EB)             # [ch, p, r, b, sl]
        h_edges = np.ascontiguousarray(
            G.transpose(3, 2, 1, 0, 4)).reshape(cfg.NB * 9, P, 2 * cfg.EB)

        in_maps.append({
            "h_edges": h_edges,
            "w_kv": w_kv,
            "w_q": w_q,
            "jac": jac.astype(bf16),
        })
    return in_maps


def assemble_output(cfg: Cfg, results):
    out = np.empty((cfg.N, NUM_HEADS, OUT_DIM), dtype=np.float32)
    for c, r in enumerate(results):
        shard = r["out"].reshape(P, cfg.M, 256).transpose(1, 0, 2).reshape(
            cfg.NPAD, 256)[:cfg.NPC]
        # un-permute: column d*8+h holds (head h, dim d)
        out[c * cfg.NPC:(c + 1) * cfg.NPC] = shard.reshape(
            cfg.NPC, OUT_DIM, NUM_HEADS).transpose(0, 2, 1)
    return out


# --------------------------------------------------------------------------
# numpy fallback (used only if inputs don't match the spec'd structure)
# --------------------------------------------------------------------------

def _numpy_reference(h, src, dst, jaccard, Wq, bq, Wk, bk, Wv, bv):
    N = h.shape[0]
    E = src.shape[0]
    h = h.astype(np.float32)
    Qh = (h @ Wq.T + bq).reshape(N, H2, OUT_DIM)
    Kh = (h @ Wk.T + bk).reshape(N, H2, OUT_DIM)
    Vh = (h @ Wv.T + bv).reshape(N, NUM_HEADS, OUT_DIM)
    score = np.sum(Kh[src] * Qh[dst], axis=-1, keepdims=True)
    score = np.exp(np.clip(score / np.sqrt(np.float32(OUT_DIM)), -5.0, 5.0))
    jac = np.broadcast_to(jaccard[:, None, None], (E, H2, 1))
    score_new = np.concatenate([score, jac], axis=1).astype(np.float32)
    contrib = (Vh[src] * score_new).astype(np.float32)
    wV = np.zeros((N, NUM_HEADS, OUT_DIM), dtype=np.float32)
    z = np.zeros((N, NUM_HEADS, 1), dtype=np.float32)
    np.add.at(wV, dst, contrib)
    np.add.at(z, dst, score_new)
    return wV / z


# --------------------------------------------------------------------------
# entry point
# --------------------------------------------------------------------------

_PROGRAM_CACHE = {}


def _get_program(cfg: Cfg):
    key = (cfg.N, cfg.E, cfg.NC)
    if key not in _PROGRAM_CACHE:
        _PROGRAM_CACHE[key] = build_program(cfg)
    return _PROGRAM_CACHE[key]


def _structure_ok(h, src, dst, jaccard, Wq, bq, Wk, bk, Wv, bv):
    if h.shape != (N_NODES, IN_DIM) or src.shape != (N_EDGES,):
        return False
    if Wq.shape != (H2 * OUT_DIM, IN_DIM) or Wv.shape != (NUM_HEADS * OUT_DIM, IN_DIM):
        return False
    if np.any(bq) or np.any(bk) or np.any(bv):
        return False
    if not np.array_equal(
            np.asarray(dst, dtype=np.int64),
            np.arange(N_EDGES, dtype=np.int64) % N_NODES):
        return False
    if src.min() < 0 or src.max() >= N_NODES:
        return False
    return True


def run_on_hw(inputs):
    from concourse.bass2jax import run_bass_via_pjrt

    cfg = FULL_CFG
    nc = _get_program(cfg)
    in_maps = host_prepare(
        cfg, inputs["h"], inputs["src"], inputs["jaccard"],
        inputs["Wq"], inputs["Wk"], inputs["Wv"])
    results = run_bass_via_pjrt(nc, in_maps, n_cores=cfg.NC)
    return assemble_output(cfg, results), results


def kernel(**inputs) -> np.ndarray:
    args = {k: np.asarray(v) for k, v in inputs.items()}
    if not _structure_ok(**args):
        return _numpy_reference(**args)
    out, _ = run_on_hw(args)
    return out


if __name__ == "__main__":
    print("building full program...")
    nc = _get_program(FULL_CFG)
    print("ok")

